# revision 1
# baseline (speedup 1.0000x reference)
"""Trainium2 Bass kernel for CuteInferMLP: E = gelu(X @ W0^T + b0) @ W1^T + b1.

Full shapes: x (2, 2048, 2048) f32, W0 (8192, 2048), b0 (8192,),
W1 (2048, 8192), b1 (2048,). Output (2, 2048, 2048) f16.

Sharding: 8-way data-parallel over the 4096 tokens (512 tokens/core).
Each core holds the full (fp16-cast) weights and computes its token
slice end to end; the host just concatenates the 8 slices.

Device layout per core (all matmuls keep weights stationary on the PE,
contraction dim on partitions):
  GEMM0: D^T[n,m] += W0T[h,n]^T-tile-stationary @ X^T[h,m]   (h = 16 k-tiles)
  act:   D^T = gelu(D^T + b0)  (ScalarE, fused bias + fp16 cast)
  GEMM1: E^T[hh,m] += W1T[n,hh]-stationary @ D^T[n,m]        (n = 64 k-tiles)
  act:   E^T = E^T + b1 (Identity activation, fp16 cast), DMA out.
"""

import numpy as np

from concourse import bacc, tile, mybir
from concourse.bass_utils import run_bass_kernel_spmd

P = 128
N_CORES = 8
B, L, H, N = 2, 2048, 2048, 8192
M = B * L                 # 4096 tokens
M_CORE = M // N_CORES     # 512 tokens per core
KB0 = H // P              # 16  k-tiles in GEMM0 (contraction over H)
NB = N // P               # 64  n-blocks (GEMM0 output partitions)
KB1 = N // P              # 64  k-tiles in GEMM1 (contraction over N)
HB = H // P               # 16  output blocks (GEMM1 output partitions)

TRACE = False             # set True by test harness for NTFF profiling
LAST_EXEC_NS = None       # populated when TRACE

_CACHED = {}


def _build_nc():
    fp16 = mybir.dt.float16
    f32 = mybir.dt.float32
    gelu = mybir.ActivationFunctionType.Gelu
    ident = mybir.ActivationFunctionType.Identity

    nc = bacc.Bacc("TRN2", target_bir_lowering=False, debug=False,
                   num_devices=N_CORES)
    xT = nc.declare_dram_parameter("xT", [P, KB0, M_CORE], fp16, isOutput=False)
    w0 = nc.declare_dram_parameter("w0", [NB, P, KB0, P], fp16, isOutput=False)
    w1 = nc.declare_dram_parameter("w1", [HB, P, KB1, P], fp16, isOutput=False)
    b0 = nc.declare_dram_parameter("b0", [P, NB], f32, isOutput=False)
    b1 = nc.declare_dram_parameter("b1", [P, HB], f32, isOutput=False)
    out = nc.declare_dram_parameter("out", [HB, P, M_CORE], fp16, isOutput=True)

    with tile.TileContext(nc) as tc:
        with (
            tc.tile_pool(name="const", bufs=1) as const_pool,
            tc.tile_pool(name="xp", bufs=1) as x_pool,
            tc.tile_pool(name="dp", bufs=1) as d_pool,
            tc.tile_pool(name="w0p", bufs=4) as w0_pool,
            tc.tile_pool(name="w1p", bufs=3) as w1_pool,
            tc.tile_pool(name="op", bufs=4) as o_pool,
            tc.tile_pool(name="psp", bufs=4, space="PSUM") as ps_pool,
        ):
            # All DMAs issue from the sync engine: its program order plus the
            # pool-slot flow control paces the weight stream so transfers
            # never flood HBM ahead of what the PE consumes next.  The x
            # tensor and the first w0 block are split into chunks and
            # interleaved so the first matmul's dependencies (x chunk 0 +
            # w0[0] chunk 0, ~260KB) land ~1.5us after DMA start instead of
            # waiting on the full 2.6MB.
            x_sb = x_pool.tile([P, KB0, M_CORE], fp16)
            d_sb = d_pool.tile([P, KB1, M_CORE], fp16)
            w0_first = w0_pool.tile([P, KB0, P], fp16, tag="w0_sb")
            XCH = 4
            KCH = KB0 // XCH
            for ch in range(XCH):
                k0, k1c = ch * KCH, (ch + 1) * KCH
                nc.sync.dma_start(out=x_sb[:, k0:k1c, :], in_=xT[:, k0:k1c, :])
                nc.sync.dma_start(
                    out=w0_first[:, k0:k1c, :], in_=w0[0, :, k0:k1c, :])
            b0_sb = const_pool.tile([P, NB], f32)
            nc.sync.dma_start(out=b0_sb[:], in_=b0[:])
            b1_sb = const_pool.tile([P, HB], f32)
            nc.sync.dma_start(out=b1_sb[:], in_=b1[:])

            # GEMM0 + bias + gelu -> D^T resident in SBUF
            for nb in range(NB):
                if nb == 0:
                    w0_sb = w0_first
                else:
                    w0_sb = w0_pool.tile([P, KB0, P], fp16, tag="w0_sb")
                    nc.sync.dma_start(out=w0_sb[:], in_=w0[nb])
                ps = ps_pool.tile([P, M_CORE], f32)
                for kb in range(KB0):
                    nc.tensor.matmul(
                        ps[:],
                        lhsT=w0_sb[:, kb, :],
                        rhs=x_sb[:, kb, :],
                        start=(kb == 0),
                        stop=(kb == KB0 - 1),
                    )
                nc.scalar.activation(
                    d_sb[:, nb, :], ps[:], gelu,
                    bias=b0_sb[:, nb:nb + 1], scale=1.0,
                )

            # GEMM1 + bias -> E^T, streamed out
            for hb in range(HB):
                w1_sb = w1_pool.tile([P, KB1, P], fp16)
                nc.sync.dma_start(out=w1_sb[:], in_=w1[hb])
                ps = ps_pool.tile([P, M_CORE], f32)
                for kb in range(KB1):
                    nc.tensor.matmul(
                        ps[:],
                        lhsT=w1_sb[:, kb, :],
                        rhs=d_sb[:, kb, :],
                        start=(kb == 0),
                        stop=(kb == KB1 - 1),
                    )
                o_sb = o_pool.tile([P, M_CORE], fp16)
                nc.scalar.activation(
                    o_sb[:], ps[:], ident,
                    bias=b1_sb[:, hb:hb + 1], scale=1.0,
                )
                nc.sync.dma_start(out=out[hb], in_=o_sb[:])

    nc.compile()
    return nc


def kernel(x, W0, bias0, W1, bias1):
    global LAST_EXEC_NS

    if "nc" not in _CACHED:
        _CACHED["nc"] = _build_nc()
    nc = _CACHED["nc"]

    x, W0, bias0, W1, bias1 = (
        np.asarray(t) for t in (x, W0, bias0, W1, bias1))
    X = np.ascontiguousarray(x.reshape(M, H)).astype(np.float16)
    w0_host = np.ascontiguousarray(
        W0.astype(np.float16).reshape(NB, P, KB0, P).transpose(0, 3, 2, 1))
    w1_host = np.ascontiguousarray(
        W1.astype(np.float16).reshape(HB, P, KB1, P).transpose(0, 3, 2, 1))
    b0_host = np.ascontiguousarray(bias0.astype(np.float32).reshape(NB, P).T)
    b1_host = np.ascontiguousarray(bias1.astype(np.float32).reshape(HB, P).T)

    in_maps = []
    for c in range(N_CORES):
        xs = X[c * M_CORE:(c + 1) * M_CORE]          # (512, 2048)
        xT_host = np.ascontiguousarray(
            xs.T.reshape(KB0, P, M_CORE).transpose(1, 0, 2))
        in_maps.append({
            "xT": xT_host, "w0": w0_host, "w1": w1_host,
            "b0": b0_host, "b1": b1_host,
        })

    res = run_bass_kernel_spmd(
        nc, in_maps, core_ids=list(range(N_CORES)), trace=TRACE)
    if TRACE:
        LAST_EXEC_NS = res.exec_time_ns

    E = np.empty((M, H), dtype=np.float16)
    for c in range(N_CORES):
        o = res.results[c]["out"]                    # (HB, P, M_CORE)
        E[c * M_CORE:(c + 1) * M_CORE] = o.transpose(2, 0, 1).reshape(M_CORE, H)
    return E.reshape(B, L, H)



# revision 4
# speedup vs baseline: 1.1815x; 1.1815x over previous
"""Trainium2 Bass kernel for CuteInferMLP: E = gelu(X @ W0^T + b0) @ W1^T + b1.

Full shapes: x (2, 2048, 2048) f32, W0 (8192, 2048), b0 (8192,),
W1 (2048, 8192), b1 (2048,). Output (2, 2048, 2048) f16.

Sharding: 8-way data-parallel over the 4096 tokens (512 tokens/core).
Each core holds the full (fp16-cast) weights and computes its token
slice end to end; the host just concatenates the 8 slices.

Device layout per core (all matmuls keep weights stationary on the PE,
contraction dim on partitions):
  GEMM0: D^T[n,m] += W0T[h,n]^T-tile-stationary @ X^T[h,m]   (h = 16 k-tiles)
  act:   D^T = gelu(D^T + b0)  (ScalarE, fused bias + fp16 cast)
  GEMM1: E^T[hh,m] += W1T[n,hh]-stationary @ D^T[n,m]        (n = 64 k-tiles)
  act:   E^T = E^T + b1 (Identity activation, fp16 cast), DMA out.
"""

import numpy as np

from concourse import bacc, tile, mybir
from concourse.bass_utils import run_bass_kernel_spmd

P = 128
N_CORES = 8
B, L, H, N = 2, 2048, 2048, 8192
M = B * L                 # 4096 tokens
M_CORE = M // N_CORES     # 512 tokens per core
KB0 = H // P              # 16  k-tiles in GEMM0 (contraction over H)
NB = N // P               # 64  n-blocks (GEMM0 output partitions)
KB1 = N // P              # 64  k-tiles in GEMM1 (contraction over N)
HB = H // P               # 16  output blocks (GEMM1 output partitions)

TRACE = False             # set True by test harness for NTFF profiling
LAST_EXEC_NS = None       # populated when TRACE

_CACHED = {}


def _build_nc():
    fp16 = mybir.dt.float16
    f32 = mybir.dt.float32
    gelu = mybir.ActivationFunctionType.Gelu
    ident = mybir.ActivationFunctionType.Identity

    nc = bacc.Bacc("TRN2", target_bir_lowering=False, debug=False,
                   num_devices=N_CORES)
    xT = nc.declare_dram_parameter("xT", [P, KB0, M_CORE], fp16, isOutput=False)
    w0 = nc.declare_dram_parameter("w0", [NB, P, KB0, P], fp16, isOutput=False)
    w1 = nc.declare_dram_parameter("w1", [HB, P, KB1, P], fp16, isOutput=False)
    b0 = nc.declare_dram_parameter("b0", [P, NB], f32, isOutput=False)
    b1 = nc.declare_dram_parameter("b1", [P, HB], f32, isOutput=False)
    out = nc.declare_dram_parameter("out", [HB, P, M_CORE], fp16, isOutput=True)

    with tile.TileContext(nc) as tc:
        with (
            tc.tile_pool(name="const", bufs=1) as const_pool,
            tc.tile_pool(name="xp", bufs=1) as x_pool,
            tc.tile_pool(name="dp", bufs=1) as d_pool,
            tc.tile_pool(name="w0p", bufs=4) as w0_pool,
            tc.tile_pool(name="w1p", bufs=3) as w1_pool,
            tc.tile_pool(name="op", bufs=4) as o_pool,
            tc.tile_pool(name="psp", bufs=6, space="PSUM") as ps_pool,
        ):
            # All DMAs issue from the sync engine: its program order plus the
            # pool-slot flow control paces the weight stream so transfers
            # never flood HBM ahead of what the PE consumes next.  Tile-level
            # dependency tracking is per-tile, so the lead-in x / first-w0
            # transfers use SEPARATE per-chunk tiles: the first matmul then
            # waits only on chunk 0 (~2.5us) instead of every chunk DMA.
            XCH = 8
            KCH = KB0 // XCH          # 2 k-tiles per chunk
            d_sb = d_pool.tile([P, KB1, M_CORE], fp16)
            x_chunks = []
            w0_chunks = []
            for ch in range(XCH):
                xc = x_pool.tile([P, KCH, M_CORE], fp16, name=f"x_c{ch}")
                wc = x_pool.tile([P, KCH, P], fp16, name=f"w0f_c{ch}")
                x_chunks.append(xc)
                w0_chunks.append(wc)
            for ch in range(XCH):
                k0, k1c = ch * KCH, (ch + 1) * KCH
                nc.sync.dma_start(out=x_chunks[ch][:], in_=xT[:, k0:k1c, :])
                nc.sync.dma_start(
                    out=w0_chunks[ch][:], in_=w0[0, :, k0:k1c, :])
            b0_sb = const_pool.tile([P, NB], f32)
            nc.sync.dma_start(out=b0_sb[:], in_=b0[:])
            b1_sb = const_pool.tile([P, HB], f32)
            nc.sync.dma_start(out=b1_sb[:], in_=b1[:])
            # GEMM0 + bias + gelu -> D^T resident in SBUF.  w1[0] is
            # prefetched a few blocks in (after the lead-in chunks have
            # landed) so the GEMM0->GEMM1 transition doesn't stall on its
            # 2MB transfer.
            w1_first = None
            for nb in range(NB):
                if nb != 0:
                    w0_sb = w0_pool.tile([P, KB0, P], fp16, tag="w0_sb")
                    nc.sync.dma_start(out=w0_sb[:], in_=w0[nb])
                if nb == 3:
                    w1_first = w1_pool.tile([P, KB1, P], fp16, tag="w1_sb")
                    nc.sync.dma_start(out=w1_first[:], in_=w1[0])
                ps = ps_pool.tile([P, M_CORE], f32)
                for kb in range(KB0):
                    if nb == 0:
                        lhs = w0_chunks[kb // KCH][:, kb % KCH, :]
                    else:
                        lhs = w0_sb[:, kb, :]
                    nc.tensor.matmul(
                        ps[:],
                        lhsT=lhs,
                        rhs=x_chunks[kb // KCH][:, kb % KCH, :],
                        start=(kb == 0),
                        stop=(kb == KB0 - 1),
                    )
                nc.scalar.activation(
                    d_sb[:, nb, :], ps[:], gelu,
                    bias=b0_sb[:, nb:nb + 1], scale=1.0,
                )

            # GEMM1 + bias -> E^T, streamed out
            for hb in range(HB):
                if hb == 0:
                    w1_sb = w1_first
                else:
                    w1_sb = w1_pool.tile([P, KB1, P], fp16, tag="w1_sb")
                    nc.sync.dma_start(out=w1_sb[:], in_=w1[hb])
                ps = ps_pool.tile([P, M_CORE], f32)
                for kb in range(KB1):
                    nc.tensor.matmul(
                        ps[:],
                        lhsT=w1_sb[:, kb, :],
                        rhs=d_sb[:, kb, :],
                        start=(kb == 0),
                        stop=(kb == KB1 - 1),
                    )
                o_sb = o_pool.tile([P, M_CORE], fp16)
                nc.scalar.activation(
                    o_sb[:], ps[:], ident,
                    bias=b1_sb[:, hb:hb + 1], scale=1.0,
                )
                nc.sync.dma_start(out=out[hb], in_=o_sb[:])

    nc.compile()
    return nc


def kernel(x, W0, bias0, W1, bias1):
    global LAST_EXEC_NS

    if "nc" not in _CACHED:
        _CACHED["nc"] = _build_nc()
    nc = _CACHED["nc"]

    x, W0, bias0, W1, bias1 = (
        np.asarray(t) for t in (x, W0, bias0, W1, bias1))
    X = np.ascontiguousarray(x.reshape(M, H)).astype(np.float16)
    w0_host = np.ascontiguousarray(
        W0.astype(np.float16).reshape(NB, P, KB0, P).transpose(0, 3, 2, 1))
    w1_host = np.ascontiguousarray(
        W1.astype(np.float16).reshape(HB, P, KB1, P).transpose(0, 3, 2, 1))
    b0_host = np.ascontiguousarray(bias0.astype(np.float32).reshape(NB, P).T)
    b1_host = np.ascontiguousarray(bias1.astype(np.float32).reshape(HB, P).T)

    in_maps = []
    for c in range(N_CORES):
        xs = X[c * M_CORE:(c + 1) * M_CORE]          # (512, 2048)
        xT_host = np.ascontiguousarray(
            xs.T.reshape(KB0, P, M_CORE).transpose(1, 0, 2))
        in_maps.append({
            "xT": xT_host, "w0": w0_host, "w1": w1_host,
            "b0": b0_host, "b1": b1_host,
        })

    res = run_bass_kernel_spmd(
        nc, in_maps, core_ids=list(range(N_CORES)), trace=TRACE)
    if TRACE:
        LAST_EXEC_NS = res.exec_time_ns

    E = np.empty((M, H), dtype=np.float16)
    for c in range(N_CORES):
        o = res.results[c]["out"]                    # (HB, P, M_CORE)
        E[c * M_CORE:(c + 1) * M_CORE] = o.transpose(2, 0, 1).reshape(M_CORE, H)
    return E.reshape(B, L, H)



# revision 6
# speedup vs baseline: 1.1966x; 1.0128x over previous
"""Trainium2 Bass kernel for CuteInferMLP: E = gelu(X @ W0^T + b0) @ W1^T + b1.

Full shapes: x (2, 2048, 2048) f32, W0 (8192, 2048), b0 (8192,),
W1 (2048, 8192), b1 (2048,). Output (2, 2048, 2048) f16.

Sharding: 8-way data-parallel over the 4096 tokens (512 tokens/core).
Each core holds the full (fp16-cast) weights and computes its token
slice end to end; the host just concatenates the 8 slices.

Device layout per core (all matmuls keep weights stationary on the PE,
contraction dim on partitions):
  GEMM0: D^T[n,m] += W0T[h,n]^T-tile-stationary @ X^T[h,m]   (h = 16 k-tiles)
  act:   D^T = gelu(D^T + b0)  (ScalarE, fused bias + fp16 cast)
  GEMM1: E^T[hh,m] += W1T[n,hh]-stationary @ D^T[n,m]        (n = 64 k-tiles)
  act:   E^T = E^T + b1 (Identity activation, fp16 cast), DMA out.
"""

import numpy as np

from concourse import bacc, tile, mybir
from concourse.bass_utils import run_bass_kernel_spmd

P = 128
N_CORES = 8
B, L, H, N = 2, 2048, 2048, 8192
M = B * L                 # 4096 tokens
M_CORE = M // N_CORES     # 512 tokens per core
KB0 = H // P              # 16  k-tiles in GEMM0 (contraction over H)
NB = N // P               # 64  n-blocks (GEMM0 output partitions)
KB1 = N // P              # 64  k-tiles in GEMM1 (contraction over N)
HB = H // P               # 16  output blocks (GEMM1 output partitions)

TRACE = False             # set True by test harness for NTFF profiling
LAST_EXEC_NS = None       # populated when TRACE

_CACHED = {}


def _build_nc():
    fp16 = mybir.dt.float16
    f32 = mybir.dt.float32
    gelu = mybir.ActivationFunctionType.Gelu
    ident = mybir.ActivationFunctionType.Identity

    nc = bacc.Bacc("TRN2", target_bir_lowering=False, debug=False,
                   num_devices=N_CORES)
    xT = nc.declare_dram_parameter("xT", [P, KB0, M_CORE], fp16, isOutput=False)
    w0 = nc.declare_dram_parameter("w0", [NB, P, KB0, P], fp16, isOutput=False)
    w1 = nc.declare_dram_parameter("w1", [HB, P, KB1, P], fp16, isOutput=False)
    b0 = nc.declare_dram_parameter("b0", [P, NB], f32, isOutput=False)
    b1 = nc.declare_dram_parameter("b1", [P, HB], f32, isOutput=False)
    out = nc.declare_dram_parameter("out", [HB, P, M_CORE], fp16, isOutput=True)

    with tile.TileContext(nc) as tc:
        with (
            tc.tile_pool(name="const", bufs=1) as const_pool,
            tc.tile_pool(name="xp", bufs=1) as x_pool,
            tc.tile_pool(name="dp", bufs=1) as d_pool,
            tc.tile_pool(name="w0p", bufs=4) as w0_pool,
            tc.tile_pool(name="w1p", bufs=3) as w1_pool,
            tc.tile_pool(name="op", bufs=4) as o_pool,
            tc.tile_pool(name="psp", bufs=6, space="PSUM") as ps_pool,
        ):
            # All DMAs issue from the sync engine: its program order plus the
            # pool-slot flow control paces the weight stream so transfers
            # never flood HBM ahead of what the PE consumes next.  The first
            # matmul needs all of x plus w0[0] (~2.5MB); splitting those
            # transfers finer only trades lead-in for stream stalls (the
            # bytes must cross HBM either way), so they go as interleaved
            # large transfers into two tiles.
            x_sb = x_pool.tile([P, KB0, M_CORE], fp16)
            d_sb = d_pool.tile([P, KB1, M_CORE], fp16)
            w0_first = w0_pool.tile([P, KB0, P], fp16, tag="w0_sb")
            XCH = 4
            KCH = KB0 // XCH
            for ch in range(XCH):
                k0, k1c = ch * KCH, (ch + 1) * KCH
                nc.sync.dma_start(out=x_sb[:, k0:k1c, :], in_=xT[:, k0:k1c, :])
                nc.sync.dma_start(
                    out=w0_first[:, k0:k1c, :], in_=w0[0, :, k0:k1c, :])
            b0_sb = const_pool.tile([P, NB], f32)
            nc.sync.dma_start(out=b0_sb[:], in_=b0[:])
            b1_sb = const_pool.tile([P, HB], f32)
            nc.sync.dma_start(out=b1_sb[:], in_=b1[:])
            # GEMM0 + bias + gelu -> D^T resident in SBUF.  w1[0] is
            # prefetched a few blocks in (after the lead-in chunks have
            # landed) so the GEMM0->GEMM1 transition doesn't stall on its
            # 2MB transfer.
            w1_first = None
            for nb in range(NB):
                if nb == 0:
                    w0_sb = w0_first
                else:
                    w0_sb = w0_pool.tile([P, KB0, P], fp16, tag="w0_sb")
                    nc.sync.dma_start(out=w0_sb[:], in_=w0[nb])
                if nb == 3:
                    w1_first = w1_pool.tile([P, KB1, P], fp16, tag="w1_sb")
                    nc.sync.dma_start(out=w1_first[:], in_=w1[0])
                ps = ps_pool.tile([P, M_CORE], f32)
                for kb in range(KB0):
                    nc.tensor.matmul(
                        ps[:],
                        lhsT=w0_sb[:, kb, :],
                        rhs=x_sb[:, kb, :],
                        start=(kb == 0),
                        stop=(kb == KB0 - 1),
                    )
                nc.scalar.activation(
                    d_sb[:, nb, :], ps[:], gelu,
                    bias=b0_sb[:, nb:nb + 1], scale=1.0,
                )

            # GEMM1 + bias -> E^T, streamed out
            for hb in range(HB):
                if hb == 0:
                    w1_sb = w1_first
                else:
                    w1_sb = w1_pool.tile([P, KB1, P], fp16, tag="w1_sb")
                    nc.sync.dma_start(out=w1_sb[:], in_=w1[hb])
                ps = ps_pool.tile([P, M_CORE], f32)
                for kb in range(KB1):
                    nc.tensor.matmul(
                        ps[:],
                        lhsT=w1_sb[:, kb, :],
                        rhs=d_sb[:, kb, :],
                        start=(kb == 0),
                        stop=(kb == KB1 - 1),
                    )
                o_sb = o_pool.tile([P, M_CORE], fp16)
                nc.scalar.activation(
                    o_sb[:], ps[:], ident,
                    bias=b1_sb[:, hb:hb + 1], scale=1.0,
                )
                nc.sync.dma_start(out=out[hb], in_=o_sb[:])

    nc.compile()
    return nc


def kernel(x, W0, bias0, W1, bias1):
    global LAST_EXEC_NS

    if "nc" not in _CACHED:
        _CACHED["nc"] = _build_nc()
    nc = _CACHED["nc"]

    x, W0, bias0, W1, bias1 = (
        np.asarray(t) for t in (x, W0, bias0, W1, bias1))
    X = np.ascontiguousarray(x.reshape(M, H)).astype(np.float16)
    w0_host = np.ascontiguousarray(
        W0.astype(np.float16).reshape(NB, P, KB0, P).transpose(0, 3, 2, 1))
    w1_host = np.ascontiguousarray(
        W1.astype(np.float16).reshape(HB, P, KB1, P).transpose(0, 3, 2, 1))
    b0_host = np.ascontiguousarray(bias0.astype(np.float32).reshape(NB, P).T)
    b1_host = np.ascontiguousarray(bias1.astype(np.float32).reshape(HB, P).T)

    in_maps = []
    for c in range(N_CORES):
        xs = X[c * M_CORE:(c + 1) * M_CORE]          # (512, 2048)
        xT_host = np.ascontiguousarray(
            xs.T.reshape(KB0, P, M_CORE).transpose(1, 0, 2))
        in_maps.append({
            "xT": xT_host, "w0": w0_host, "w1": w1_host,
            "b0": b0_host, "b1": b1_host,
        })

    res = run_bass_kernel_spmd(
        nc, in_maps, core_ids=list(range(N_CORES)), trace=TRACE)
    if TRACE:
        LAST_EXEC_NS = res.exec_time_ns

    E = np.empty((M, H), dtype=np.float16)
    for c in range(N_CORES):
        o = res.results[c]["out"]                    # (HB, P, M_CORE)
        E[c * M_CORE:(c + 1) * M_CORE] = o.transpose(2, 0, 1).reshape(M_CORE, H)
    return E.reshape(B, L, H)

